# revision 8
# baseline (speedup 1.0000x reference)
"""Trainium2 Bass kernel for the CGC multi-task MoE routing module.

Math: the reference computes, per task t:
    expert outputs  E[t,e] = x @ W[t,e] + b[t,e]          (ES specific + EC common)
    gate logits     L[t]   = concat_e(E[t,e]) @ Wg[t] + bg[t]
    weights         p      = softmax(L[t])
    feature         F[t]   = sum_e p_e * E[t,e]
    out[t]          = F[t] @ Wt[t] + bt[t]                # scalar per sample

Both L[t] and the per-expert scalars s[t,e] = E[t,e] @ Wt[t] are linear in x,
so everything folds into one skinny matmul z = x @ A + d with
A: [I, 24] (per task: 6 logit cols + 6 scalar cols), followed by a per-sample
6-way softmax-weighted average:
    out[t,b] = sum_e exp(L_e) * s_e / sum_e exp(L_e)      (bt folded into s).

v4: x and A are bf16 (tolerance 2e-2; bf16 dot error ~2e-3), halving HBM
traffic.  x loads are all issued up-front, alternating between the two
HWDGE rings (sync + scalar); A/d ride gpsimd SWDGE.  The final band is
split into 128-sample pieces so the post-stream serial chain
(matmul->bias->transpose->exp->reduce->divide->store) runs on tiny
free dims.  The softmax-average uses a fused elementwise divide.
"""

import os

import numpy as np

B, I, H = 65536, 512, 128
T, ES, EC = 2, 2, 4
ETOT = ES + EC

N_CORES = 8
BS = B // N_CORES  # samples per core
M = 32  # folded output channels, padded 24 -> 32 for the 32x32 transpose
GW = 512  # samples per band (one PSUM bank per band)
QW = 4 * GW  # samples per quad (4 bands stacked on the 128 partitions)
NQ = BS // QW
NCHUNK = I // 128
NB = GW // 32  # 32-sample blocks per band

# x loads (samples): 1MB loads for bands 0..13, then a taper so the last
# pieces complete (and can be post-processed) with minimal serial latency
LOADS = [
    (0, 1024),
    (1024, 1024),
    (2048, 1024),
    (3072, 1024),
    (4096, 1024),
    (5120, 1024),
    (6144, 1024),
    (7168, 512),
    (7680, 256),
    (7936, 128),
    (8064, 128),
]
assert sum(n for _, n in LOADS) == BS


def _fold(inputs):
    """Fold all weights into A [128, NCHUNK, M] (bf16) and bias d [M,1] (f32).

    Channel layout per task t (base 12*t): 0:6 gate logits, 6:12 per-expert
    scalars (bt folded in, valid since softmax weights sum to 1).
    A is packed so that partition p, chunk c holds row c*128+p of the
    [I, M] matrix (matching the xT chunk view).
    """
    import ml_dtypes

    w64 = lambda k: np.asarray(inputs[k], np.float64)
    Wc, bc, Ws, bs = w64("Wc"), w64("bc"), w64("Ws"), w64("bs")
    Wg, bg, Wt, bt = w64("Wg"), w64("bg"), w64("Wt"), w64("bt")

    A = np.zeros((I, M))
    d = np.zeros(M)
    for t in range(T):
        W_all = np.concatenate(
            [Ws[t, e] for e in range(ES)] + [Wc[e] for e in range(EC)], axis=1
        )  # [I, ETOT*H]
        b_all = np.concatenate(
            [bs[t, e] for e in range(ES)] + [bc[e] for e in range(EC)]
        )  # [ETOT*H]
        A[:, 12 * t : 12 * t + 6] = W_all @ Wg[t]
        d[12 * t : 12 * t + 6] = b_all @ Wg[t] + bg[t]
        A[:, 12 * t + 6 : 12 * t + 12] = (
            W_all.reshape(I, ETOT, H) * Wt[t, :, 0][None, None, :]
        ).sum(-1)
        d[12 * t + 6 : 12 * t + 12] = (
            b_all.reshape(ETOT, H) * Wt[t, :, 0][None, :]
        ).sum(-1) + bt[t, 0]
    Apack = (
        A.reshape(NCHUNK, 128, M).transpose(1, 0, 2).astype(ml_dtypes.bfloat16)
    )  # [128, NCHUNK, M]
    return np.ascontiguousarray(Apack), d.reshape(M, 1).astype(np.float32)


def _build_program():
    import concourse.bacc as bacc
    import concourse.mybir as mybir
    from concourse.tile import TileContext

    f32 = mybir.dt.float32
    bf16 = mybir.dt.bfloat16

    nc = bacc.Bacc("TRN2", target_bir_lowering=False, debug=False, num_devices=N_CORES)
    xT_ext = nc.declare_dram_parameter("xT", [I, BS], bf16, isOutput=False)
    A_ext = nc.declare_dram_parameter("A", [128, NCHUNK, M], bf16, isOutput=False)
    d_ext = nc.declare_dram_parameter("d", [M, 1], f32, isOutput=False)
    # out[q, p, blk, t]: sample s = q*QW + (p//32)*GW + 32*blk + p%32, task t
    out_ext = nc.declare_dram_parameter("out", [NQ, 128, NB, T], f32, isOutput=True)

    xT_view = xT_ext[:, :].rearrange("(c p) b -> p c b", p=128)  # [128, NCHUNK, BS]

    with TileContext(nc) as tc:
        with (
            tc.tile_pool(name="consts", bufs=1) as cpool,
            tc.tile_pool(name="xin", bufs=1) as xpool,
            tc.tile_pool(name="zt", bufs=3) as ztpool,
            tc.tile_pool(name="zq", bufs=3) as zqpool,
            tc.tile_pool(name="epi", bufs=4) as epool,
            tc.tile_pool(name="psum", bufs=2, space="PSUM") as ppool,
        ):
            # A/d ride the gpsimd SWDGE so both HWDGE rings stream x
            A_sb = cpool.tile([128, NCHUNK, M], bf16)
            nc.gpsimd.dma_start(out=A_sb[:], in_=A_ext[:, :, :])
            d_sb = cpool.tile([M, 1], f32)
            nc.gpsimd.dma_start(out=d_sb[:], in_=d_ext[:, :])

            # prefetch the whole shard, alternating the two HWDGE rings
            xs = []
            for k, (s0, n) in enumerate(LOADS):
                xk = xpool.tile([128, NCHUNK, n], bf16, name=f"x_{k}", tag=f"x{k}")
                eng = nc.sync if k % 2 == 0 else nc.scalar
                eng.dma_start(out=xk[:], in_=xT_view[:, :, s0 : s0 + n])
                xs.append(xk)

            # band b (0..14) -> (load tile, sample offset within tile)
            def band_src(b):
                if b < 14:
                    return xs[b // 2], (b % 2) * GW
                return xs[7], 0  # band 14

            # piece p (0..3) of band 15 -> (load tile, offset)
            def piece_src(p):
                return [(xs[8], 0), (xs[8], 128), (xs[9], 0), (xs[10], 0)][p]

            from collections import deque

            pending = deque()

            def emit(fn):
                pending.append(fn)
                if len(pending) > 1:
                    pending.popleft()()

            ActT = mybir.ActivationFunctionType
            AxX = mybir.AxisListType.X
            AluAdd = mybir.AluOpType.add
            AluDiv = mybir.AluOpType.divide

            def mm_band(ps_ap, src, c_outer_items=None, *, cols):
                pass  # unused helper placeholder

            def epilogue(idx, Z, npart, nblk, res_t, store_fn):
                """softmax-weighted average on Z [npart, nblk*32]; writes
                res_t [npart, nblk, T]; then store_fn() if given."""
                Zb = Z.rearrange("p (blk c) -> p blk c", c=32)
                zt4 = Zb[:, :, 0:24].rearrange("p blk (t c) -> p blk t c", c=12)
                lg = zt4[:, :, :, 0:6]
                sc = zt4[:, :, :, 6:12]
                ssum = epool.tile(
                    [npart, nblk, T], f32, name=f"ssum_{idx}", tag="ssum"
                )
                num = epool.tile([npart, nblk, T], f32, name=f"num_{idx}", tag="num")
                rinv = epool.tile(
                    [npart, nblk, T], f32, name=f"rinv_{idx}", tag="rinv"
                )

                nc.scalar.activation(lg, lg, ActT.Exp)
                nc.vector.tensor_reduce(ssum[:], lg, axis=AxX, op=AluAdd)
                nc.vector.tensor_mul(lg, lg, sc)  # exp * s, clobbers exp
                nc.vector.tensor_reduce(num[:], lg, axis=AxX, op=AluAdd)
                nc.vector.reciprocal(rinv[:], ssum[:])
                nc.vector.tensor_mul(res_t, num[:], rinv[:])
                if store_fn is not None:
                    store_fn()

            # ---- quads 0..2: full 128-partition pipeline ----
            for q in range(3):
                zT_sb = ztpool.tile([128, GW], f32, name=f"zTsb_{q}", tag="zTsb")
                pss = [
                    ppool.tile([M, GW], f32, name=f"ps_{q}_{j}", tag=f"ps{j}")
                    for j in range(4)
                ]
                # chunk-outer: consecutive matmuls share stationary weights
                for c in range(NCHUNK):
                    for j in range(4):
                        xk, off = band_src(4 * q + j)
                        nc.tensor.matmul(
                            pss[j][:, :],
                            A_sb[:, c, :],
                            xk[:, c, off : off + GW],
                            start=(c == 0),
                            stop=(c == NCHUNK - 1),
                        )
                # PSUM -> SBUF band writes with per-partition bias add,
                # split across scalar and vector (gpsimd can't read PSUM)
                for j in range(4):
                    dst = zT_sb[32 * j : 32 * j + 32, :]
                    if j < 3:
                        nc.scalar.add(dst, pss[j][:, :], d_sb[:])
                    else:
                        nc.vector.tensor_scalar_add(dst, pss[j][:, :], d_sb[:])

                Zq = zqpool.tile([128, GW], f32, name=f"Z_{q}", tag="Z")
                nc.vector.transpose(Zq[:], zT_sb[:])

                def epi_quad(q=q, Zq=Zq):
                    res = epool.tile([128, NB, T], f32, name=f"res_{q}", tag="res")
                    epilogue(
                        f"q{q}",
                        Zq[:],
                        128,
                        NB,
                        res[:],
                        lambda: nc.sync.dma_start(
                            out=out_ext[q, :, :, :], in_=res[:]
                        ),
                    )

                emit(epi_quad)

            # ---- bands 12,13: 64-partition group ----
            zT34 = ztpool.tile([64, GW], f32, name="zT34", tag="zt34", bufs=1)
            ps34 = [
                ppool.tile([M, GW], f32, name=f"ps34_{j}", tag=f"ps{j}")
                for j in range(2)
            ]
            for j in range(2):
                xk, off = band_src(12 + j)
                for c in range(NCHUNK):
                    nc.tensor.matmul(
                        ps34[j][:, :],
                        A_sb[:, c, :],
                        xk[:, c, off : off + GW],
                        start=(c == 0),
                        stop=(c == NCHUNK - 1),
                    )
                dst = zT34[32 * j : 32 * j + 32, :]
                if j == 0:
                    nc.scalar.add(dst, ps34[j][:, :], d_sb[:])
                else:
                    nc.vector.tensor_scalar_add(dst, ps34[j][:, :], d_sb[:])
            Zq34 = zqpool.tile([64, GW], f32, name="Zq34", tag="z34", bufs=1)
            nc.vector.transpose(Zq34[:], zT34[:])

            def epi_34(Zq34=Zq34):
                res = epool.tile([64, NB, T], f32, name="res_34", tag="res")
                epilogue(
                    "g34",
                    Zq34[:],
                    64,
                    NB,
                    res[:],
                    lambda: nc.sync.dma_start(
                        out=out_ext[3, 0:64, :, :], in_=res[:]
                    ),
                )

            emit(epi_34)

            # ---- band 14: 32-partition group ----
            zT14 = ztpool.tile([M, GW], f32, name="zT14", tag="zt14", bufs=1)
            ps14 = ppool.tile([M, GW], f32, name="ps14", tag="ps2")
            xk, off = band_src(14)
            for c in range(NCHUNK):
                nc.tensor.matmul(
                    ps14[:, :],
                    A_sb[:, c, :],
                    xk[:, c, off : off + GW],
                    start=(c == 0),
                    stop=(c == NCHUNK - 1),
                )
            nc.scalar.add(zT14[:, :], ps14[:, :], d_sb[:])
            Zq14 = zqpool.tile([M, GW], f32, name="Zq14", tag="z14", bufs=1)
            nc.vector.transpose(Zq14[:], zT14[:])

            def epi_14(Zq14=Zq14):
                res = epool.tile([M, NB, T], f32, name="res_14", tag="res")
                epilogue(
                    "g14",
                    Zq14[:],
                    32,
                    NB,
                    res[:],
                    lambda: nc.sync.dma_start(
                        out=out_ext[3, 64:96, :, :], in_=res[:]
                    ),
                )

            emit(epi_14)

            # ---- band 15: four 128-sample pieces (short tail chain) ----
            PW = 128  # piece width
            zT15 = ztpool.tile([M, GW], f32, name="zT15", tag="zt15", bufs=1)
            ps15 = ppool.tile([M, GW], f32, name="ps15", tag="ps3")
            Zq15 = zqpool.tile([M, GW], f32, name="Zq15", tag="z15", bufs=1)
            res15 = epool.tile([M, NB, T], f32, name="res_15", tag="res15", bufs=1)
            for p in range(4):
                xk, off = piece_src(p)
                pslice = ps15[:, PW * p : PW * (p + 1)]
                for c in range(NCHUNK):
                    nc.tensor.matmul(
                        pslice,
                        A_sb[:, c, :],
                        xk[:, c, off : off + PW],
                        start=(c == 0),
                        stop=(c == NCHUNK - 1),
                    )
                dst = zT15[:, PW * p : PW * (p + 1)]
                if p % 2 == 0:
                    nc.scalar.add(dst, pslice, d_sb[:])
                else:
                    nc.vector.tensor_scalar_add(dst, pslice, d_sb[:])
                nc.vector.transpose(
                    Zq15[:, PW * p : PW * (p + 1)], zT15[:, PW * p : PW * (p + 1)]
                )

                def epi_piece(p=p):
                    store = None
                    if p == 3:
                        store = lambda: nc.sync.dma_start(
                            out=out_ext[3, 96:128, :, :], in_=res15[:]
                        )
                    epilogue(
                        f"p{p}",
                        Zq15[:, PW * p : PW * (p + 1)],
                        32,
                        PW // 32,
                        res15[:, 4 * p : 4 * (p + 1), :],
                        store,
                    )

                emit(epi_piece)

            while pending:
                pending.popleft()()

    nc.compile()
    return nc


_PROGRAM = None


def _ensure_ntff_hook():
    """Provide antenv.axon_hooks if the image lacks it (NTFF profiling)."""
    try:
        import antenv.axon_hooks  # noqa: F401

        return
    except ImportError:
        pass
    import contextlib
    import ctypes
    import sys
    import types

    import antenv

    mod = types.ModuleType("antenv.axon_hooks")
    holder = {"hook": None}
    mod.set_axon_ntff_profile_hook = lambda h: holder.__setitem__("hook", h)
    mod.get_axon_ntff_profile_hook = lambda: holder["hook"]
    sys.modules["antenv.axon_hooks"] = mod
    antenv.axon_hooks = mod

    so_path = "/opt/axon/libaxon_pjrt.so"
    try:
        lib = ctypes.CDLL(so_path)
    except OSError:
        return
    if not hasattr(lib, "axon_start_nrt_profile"):
        return
    lib.axon_start_nrt_profile.argtypes = [
        ctypes.POINTER(ctypes.c_int64),
        ctypes.c_size_t,
    ]
    lib.axon_start_nrt_profile.restype = ctypes.c_int64
    lib.axon_stop_nrt_profile.argtypes = [ctypes.c_char_p]
    lib.axon_stop_nrt_profile.restype = ctypes.c_int64

    @contextlib.contextmanager
    def _hook(output_dir, device_ids):
        import jax

        jax.devices()
        if device_ids:
            ids = (ctypes.c_int64 * len(device_ids))(*device_ids)
            rc = lib.axon_start_nrt_profile(ids, len(device_ids))
        else:
            rc = lib.axon_start_nrt_profile(None, 0)
        if rc != 0:
            raise RuntimeError(f"axon_start_nrt_profile rc={rc}")
        try:
            yield
        finally:
            n = lib.axon_stop_nrt_profile(str(output_dir).encode())
            print(f"ntff profile: {n} file(s) written to {output_dir}")

    mod.set_axon_ntff_profile_hook(_hook)


def _run(inputs, trace=False):
    global _PROGRAM
    import ml_dtypes

    import concourse.bass_utils as bass_utils

    if trace:
        _ensure_ntff_hook()
        # keep trace artifacts local; no bucket in this sandbox
        bass_utils.upload_artifacts = lambda tmpdir: "local://" + tmpdir

    A, d = _fold(inputs)
    x = np.asarray(inputs["x"], np.float32)
    in_maps = []
    for i in range(N_CORES):
        shard_T = np.ascontiguousarray(
            x[i * BS : (i + 1) * BS].T.astype(ml_dtypes.bfloat16)
        )  # [I, BS] bf16
        in_maps.append({"xT": shard_T, "A": A, "d": d})

    if _PROGRAM is None:
        _PROGRAM = _build_program()

    kres = bass_utils.run_bass_kernel_spmd(
        _PROGRAM, in_maps, core_ids=list(range(N_CORES)), trace=trace
    )

    parts = []
    for i in range(N_CORES):
        o = np.asarray(kres.results[i]["out"])  # [NQ, 128, NB, T]
        # s = q*QW + j*GW + 32*blk + r with p = 32*j + r
        o = o.reshape(NQ, 4, 32, NB, T)  # q, j, r, blk, t
        parts.append(o.transpose(4, 0, 1, 3, 2).reshape(T, BS))
    full = np.concatenate(parts, axis=1)[:, :, None].astype(np.float32)
    return full, kres


def kernel(**inputs):
    out, _ = _run(inputs, trace=bool(int(os.environ.get("KERNEL_TRACE", "0"))))
    return out


# revision 9
# speedup vs baseline: 1.0249x; 1.0249x over previous
"""Trainium2 Bass kernel for the CGC multi-task MoE routing module.

Math: the reference computes, per task t:
    expert outputs  E[t,e] = x @ W[t,e] + b[t,e]          (ES specific + EC common)
    gate logits     L[t]   = concat_e(E[t,e]) @ Wg[t] + bg[t]
    weights         p      = softmax(L[t])
    feature         F[t]   = sum_e p_e * E[t,e]
    out[t]          = F[t] @ Wt[t] + bt[t]                # scalar per sample

Both L[t] and the per-expert scalars s[t,e] = E[t,e] @ Wt[t] are linear in x,
so everything folds into one skinny matmul z = x @ A + d with
A: [I, 24] (per task: 6 logit cols + 6 scalar cols), followed by a per-sample
6-way softmax-weighted average:
    out[t,b] = sum_e exp(L_e) * s_e / sum_e exp(L_e)      (bt folded into s).

v4: x and A are bf16 (tolerance 2e-2; bf16 dot error ~2e-3), halving HBM
traffic.  x loads are all issued up-front, alternating between the two
HWDGE rings (sync + scalar); A/d ride gpsimd SWDGE.  The final band is
split into 128-sample pieces so the post-stream serial chain
(matmul->bias->transpose->exp->reduce->divide->store) runs on tiny
free dims.  The softmax-average uses a fused elementwise divide.
"""

import os

import numpy as np

B, I, H = 65536, 512, 128
T, ES, EC = 2, 2, 4
ETOT = ES + EC

N_CORES = 8
BS = B // N_CORES  # samples per core
M = 32  # folded output channels, padded 24 -> 32 for the 32x32 transpose
GW = 512  # samples per band (one PSUM bank per band)
QW = 4 * GW  # samples per quad (4 bands stacked on the 128 partitions)
NQ = BS // QW
NCHUNK = I // 128
NB = GW // 32  # 32-sample blocks per band

# x loads (samples): 1MB loads for bands 0..13, then a taper so the last
# pieces complete (and can be post-processed) with minimal serial latency
LOADS = [
    (0, 1024),
    (1024, 1024),
    (2048, 1024),
    (3072, 1024),
    (4096, 1024),
    (5120, 1024),
    (6144, 1024),
    (7168, 512),
    (7680, 256),
    (7936, 128),
    (8064, 128),
]
assert sum(n for _, n in LOADS) == BS


def _fold(inputs):
    """Fold all weights into A [128, NCHUNK, M] (bf16) and bias d [M,1] (f32).

    Channel layout per task t (base 12*t): 0:6 gate logits, 6:12 per-expert
    scalars (bt folded in, valid since softmax weights sum to 1).
    A is packed so that partition p, chunk c holds row c*128+p of the
    [I, M] matrix (matching the xT chunk view).
    """
    import ml_dtypes

    w64 = lambda k: np.asarray(inputs[k], np.float64)
    Wc, bc, Ws, bs = w64("Wc"), w64("bc"), w64("Ws"), w64("bs")
    Wg, bg, Wt, bt = w64("Wg"), w64("bg"), w64("Wt"), w64("bt")

    A = np.zeros((I, M))
    d = np.zeros(M)
    for t in range(T):
        W_all = np.concatenate(
            [Ws[t, e] for e in range(ES)] + [Wc[e] for e in range(EC)], axis=1
        )  # [I, ETOT*H]
        b_all = np.concatenate(
            [bs[t, e] for e in range(ES)] + [bc[e] for e in range(EC)]
        )  # [ETOT*H]
        A[:, 12 * t : 12 * t + 6] = W_all @ Wg[t]
        d[12 * t : 12 * t + 6] = b_all @ Wg[t] + bg[t]
        A[:, 12 * t + 6 : 12 * t + 12] = (
            W_all.reshape(I, ETOT, H) * Wt[t, :, 0][None, None, :]
        ).sum(-1)
        d[12 * t + 6 : 12 * t + 12] = (
            b_all.reshape(ETOT, H) * Wt[t, :, 0][None, :]
        ).sum(-1) + bt[t, 0]
    Apack = (
        A.reshape(NCHUNK, 128, M).transpose(1, 0, 2).astype(ml_dtypes.bfloat16)
    )  # [128, NCHUNK, M]
    return np.ascontiguousarray(Apack), d.reshape(M, 1).astype(np.float32)


def _build_program():
    import concourse.bacc as bacc
    import concourse.mybir as mybir
    from concourse.tile import TileContext

    f32 = mybir.dt.float32
    bf16 = mybir.dt.bfloat16

    nc = bacc.Bacc("TRN2", target_bir_lowering=False, debug=False, num_devices=N_CORES)
    xT_ext = nc.declare_dram_parameter("xT", [I, BS], bf16, isOutput=False)
    A_ext = nc.declare_dram_parameter("A", [128, NCHUNK, M], bf16, isOutput=False)
    d_ext = nc.declare_dram_parameter("d", [M, 1], f32, isOutput=False)
    # out[q, p, blk, t]: sample s = q*QW + (p//32)*GW + 32*blk + p%32, task t
    out_ext = nc.declare_dram_parameter("out", [NQ, 128, NB, T], f32, isOutput=True)

    xT_view = xT_ext[:, :].rearrange("(c p) b -> p c b", p=128)  # [128, NCHUNK, BS]

    with TileContext(nc) as tc:
        with (
            tc.tile_pool(name="consts", bufs=1) as cpool,
            tc.tile_pool(name="xin", bufs=1) as xpool,
            tc.tile_pool(name="zt", bufs=3) as ztpool,
            tc.tile_pool(name="zq", bufs=3) as zqpool,
            tc.tile_pool(name="epi", bufs=4) as epool,
            tc.tile_pool(name="psum", bufs=2, space="PSUM") as ppool,
        ):
            # A/d ride the gpsimd SWDGE; scalar stays compute-only (a DMA
            # issue queued before the bias-adds would head-of-line-block them)
            A_sb = cpool.tile([128, NCHUNK, M], bf16)
            nc.gpsimd.dma_start(out=A_sb[:], in_=A_ext[:, :, :])
            d_sb = cpool.tile([M, 1], f32)
            nc.gpsimd.dma_start(out=d_sb[:], in_=d_ext[:, :])

            # prefetch the whole shard, alternating sync HWDGE and gpsimd
            # SWDGE queues so two descriptor streams feed the SDMA engines
            xs = []
            for k, (s0, n) in enumerate(LOADS):
                xk = xpool.tile([128, NCHUNK, n], bf16, name=f"x_{k}", tag=f"x{k}")
                eng = nc.sync if k % 2 == 0 else nc.gpsimd
                eng.dma_start(out=xk[:], in_=xT_view[:, :, s0 : s0 + n])
                xs.append(xk)

            # band b (0..14) -> (load tile, sample offset within tile)
            def band_src(b):
                if b < 14:
                    return xs[b // 2], (b % 2) * GW
                return xs[7], 0  # band 14

            # piece p (0..3) of band 15 -> (load tile, offset)
            def piece_src(p):
                return [(xs[8], 0), (xs[8], 128), (xs[9], 0), (xs[10], 0)][p]

            from collections import deque

            pending = deque()

            def emit(fn):
                pending.append(fn)
                if len(pending) > 1:
                    pending.popleft()()

            ActT = mybir.ActivationFunctionType
            AxX = mybir.AxisListType.X
            AluAdd = mybir.AluOpType.add
            AluDiv = mybir.AluOpType.divide

            def mm_band(ps_ap, src, c_outer_items=None, *, cols):
                pass  # unused helper placeholder

            def epilogue(idx, Z, npart, nblk, res_t, store_fn):
                """softmax-weighted average on Z [npart, nblk*32]; writes
                res_t [npart, nblk, T]; then store_fn() if given."""
                Zb = Z.rearrange("p (blk c) -> p blk c", c=32)
                zt4 = Zb[:, :, 0:24].rearrange("p blk (t c) -> p blk t c", c=12)
                lg = zt4[:, :, :, 0:6]
                sc = zt4[:, :, :, 6:12]
                ssum = epool.tile(
                    [npart, nblk, T], f32, name=f"ssum_{idx}", tag="ssum"
                )
                num = epool.tile([npart, nblk, T], f32, name=f"num_{idx}", tag="num")
                rinv = epool.tile(
                    [npart, nblk, T], f32, name=f"rinv_{idx}", tag="rinv"
                )

                nc.scalar.activation(lg, lg, ActT.Exp)
                nc.vector.tensor_reduce(ssum[:], lg, axis=AxX, op=AluAdd)
                nc.vector.tensor_mul(lg, lg, sc)  # exp * s, clobbers exp
                nc.vector.tensor_reduce(num[:], lg, axis=AxX, op=AluAdd)
                nc.vector.reciprocal(rinv[:], ssum[:])
                nc.vector.tensor_mul(res_t, num[:], rinv[:])
                if store_fn is not None:
                    store_fn()

            # ---- quads 0..2: full 128-partition pipeline ----
            for q in range(3):
                zT_sb = ztpool.tile([128, GW], f32, name=f"zTsb_{q}", tag="zTsb")
                pss = [
                    ppool.tile([M, GW], f32, name=f"ps_{q}_{j}", tag=f"ps{j}")
                    for j in range(4)
                ]
                # chunk-outer: consecutive matmuls share stationary weights
                for c in range(NCHUNK):
                    for j in range(4):
                        xk, off = band_src(4 * q + j)
                        nc.tensor.matmul(
                            pss[j][:, :],
                            A_sb[:, c, :],
                            xk[:, c, off : off + GW],
                            start=(c == 0),
                            stop=(c == NCHUNK - 1),
                        )
                # PSUM -> SBUF band writes with per-partition bias add,
                # split across scalar and vector (gpsimd can't read PSUM)
                for j in range(4):
                    dst = zT_sb[32 * j : 32 * j + 32, :]
                    if j < 3:
                        nc.scalar.add(dst, pss[j][:, :], d_sb[:])
                    else:
                        nc.vector.tensor_scalar_add(dst, pss[j][:, :], d_sb[:])

                Zq = zqpool.tile([128, GW], f32, name=f"Z_{q}", tag="Z")
                nc.vector.transpose(Zq[:], zT_sb[:])

                def epi_quad(q=q, Zq=Zq):
                    res = epool.tile([128, NB, T], f32, name=f"res_{q}", tag="res")
                    epilogue(
                        f"q{q}",
                        Zq[:],
                        128,
                        NB,
                        res[:],
                        lambda: nc.sync.dma_start(
                            out=out_ext[q, :, :, :], in_=res[:]
                        ),
                    )

                emit(epi_quad)

            # ---- bands 12,13: 64-partition group ----
            zT34 = ztpool.tile([64, GW], f32, name="zT34", tag="zt34", bufs=1)
            ps34 = [
                ppool.tile([M, GW], f32, name=f"ps34_{j}", tag=f"ps{j}")
                for j in range(2)
            ]
            for j in range(2):
                xk, off = band_src(12 + j)
                for c in range(NCHUNK):
                    nc.tensor.matmul(
                        ps34[j][:, :],
                        A_sb[:, c, :],
                        xk[:, c, off : off + GW],
                        start=(c == 0),
                        stop=(c == NCHUNK - 1),
                    )
                dst = zT34[32 * j : 32 * j + 32, :]
                if j == 0:
                    nc.scalar.add(dst, ps34[j][:, :], d_sb[:])
                else:
                    nc.vector.tensor_scalar_add(dst, ps34[j][:, :], d_sb[:])
            Zq34 = zqpool.tile([64, GW], f32, name="Zq34", tag="z34", bufs=1)
            nc.vector.transpose(Zq34[:], zT34[:])

            def epi_34(Zq34=Zq34):
                res = epool.tile([64, NB, T], f32, name="res_34", tag="res")
                epilogue(
                    "g34",
                    Zq34[:],
                    64,
                    NB,
                    res[:],
                    lambda: nc.sync.dma_start(
                        out=out_ext[3, 0:64, :, :], in_=res[:]
                    ),
                )

            emit(epi_34)

            # ---- band 14: 32-partition group ----
            zT14 = ztpool.tile([M, GW], f32, name="zT14", tag="zt14", bufs=1)
            ps14 = ppool.tile([M, GW], f32, name="ps14", tag="ps2")
            xk, off = band_src(14)
            for c in range(NCHUNK):
                nc.tensor.matmul(
                    ps14[:, :],
                    A_sb[:, c, :],
                    xk[:, c, off : off + GW],
                    start=(c == 0),
                    stop=(c == NCHUNK - 1),
                )
            nc.scalar.add(zT14[:, :], ps14[:, :], d_sb[:])
            Zq14 = zqpool.tile([M, GW], f32, name="Zq14", tag="z14", bufs=1)
            nc.vector.transpose(Zq14[:], zT14[:])

            def epi_14(Zq14=Zq14):
                res = epool.tile([M, NB, T], f32, name="res_14", tag="res")
                epilogue(
                    "g14",
                    Zq14[:],
                    32,
                    NB,
                    res[:],
                    lambda: nc.sync.dma_start(
                        out=out_ext[3, 64:96, :, :], in_=res[:]
                    ),
                )

            emit(epi_14)

            # ---- band 15: four 128-sample pieces (short tail chain) ----
            PW = 128  # piece width
            zT15 = ztpool.tile([M, GW], f32, name="zT15", tag="zt15", bufs=1)
            ps15 = ppool.tile([M, GW], f32, name="ps15", tag="ps3")
            Zq15 = zqpool.tile([M, GW], f32, name="Zq15", tag="z15", bufs=1)
            res15 = epool.tile([M, NB, T], f32, name="res_15", tag="res15", bufs=1)
            for p in range(4):
                xk, off = piece_src(p)
                pslice = ps15[:, PW * p : PW * (p + 1)]
                for c in range(NCHUNK):
                    nc.tensor.matmul(
                        pslice,
                        A_sb[:, c, :],
                        xk[:, c, off : off + PW],
                        start=(c == 0),
                        stop=(c == NCHUNK - 1),
                    )
                dst = zT15[:, PW * p : PW * (p + 1)]
                if p % 2 == 0:
                    nc.scalar.add(dst, pslice, d_sb[:])
                else:
                    nc.vector.tensor_scalar_add(dst, pslice, d_sb[:])
                nc.vector.transpose(
                    Zq15[:, PW * p : PW * (p + 1)], zT15[:, PW * p : PW * (p + 1)]
                )

                def epi_piece(p=p):
                    store = None
                    if p == 3:
                        store = lambda: nc.sync.dma_start(
                            out=out_ext[3, 96:128, :, :], in_=res15[:]
                        )
                    epilogue(
                        f"p{p}",
                        Zq15[:, PW * p : PW * (p + 1)],
                        32,
                        PW // 32,
                        res15[:, 4 * p : 4 * (p + 1), :],
                        store,
                    )

                emit(epi_piece)

            while pending:
                pending.popleft()()

    nc.compile()
    return nc


_PROGRAM = None


def _ensure_ntff_hook():
    """Provide antenv.axon_hooks if the image lacks it (NTFF profiling)."""
    try:
        import antenv.axon_hooks  # noqa: F401

        return
    except ImportError:
        pass
    import contextlib
    import ctypes
    import sys
    import types

    import antenv

    mod = types.ModuleType("antenv.axon_hooks")
    holder = {"hook": None}
    mod.set_axon_ntff_profile_hook = lambda h: holder.__setitem__("hook", h)
    mod.get_axon_ntff_profile_hook = lambda: holder["hook"]
    sys.modules["antenv.axon_hooks"] = mod
    antenv.axon_hooks = mod

    so_path = "/opt/axon/libaxon_pjrt.so"
    try:
        lib = ctypes.CDLL(so_path)
    except OSError:
        return
    if not hasattr(lib, "axon_start_nrt_profile"):
        return
    lib.axon_start_nrt_profile.argtypes = [
        ctypes.POINTER(ctypes.c_int64),
        ctypes.c_size_t,
    ]
    lib.axon_start_nrt_profile.restype = ctypes.c_int64
    lib.axon_stop_nrt_profile.argtypes = [ctypes.c_char_p]
    lib.axon_stop_nrt_profile.restype = ctypes.c_int64

    @contextlib.contextmanager
    def _hook(output_dir, device_ids):
        import jax

        jax.devices()
        if device_ids:
            ids = (ctypes.c_int64 * len(device_ids))(*device_ids)
            rc = lib.axon_start_nrt_profile(ids, len(device_ids))
        else:
            rc = lib.axon_start_nrt_profile(None, 0)
        if rc != 0:
            raise RuntimeError(f"axon_start_nrt_profile rc={rc}")
        try:
            yield
        finally:
            n = lib.axon_stop_nrt_profile(str(output_dir).encode())
            print(f"ntff profile: {n} file(s) written to {output_dir}")

    mod.set_axon_ntff_profile_hook(_hook)


def _run(inputs, trace=False):
    global _PROGRAM
    import ml_dtypes

    import concourse.bass_utils as bass_utils

    if trace:
        _ensure_ntff_hook()
        # keep trace artifacts local; no bucket in this sandbox
        bass_utils.upload_artifacts = lambda tmpdir: "local://" + tmpdir

    A, d = _fold(inputs)
    x = np.asarray(inputs["x"], np.float32)
    in_maps = []
    for i in range(N_CORES):
        shard_T = np.ascontiguousarray(
            x[i * BS : (i + 1) * BS].T.astype(ml_dtypes.bfloat16)
        )  # [I, BS] bf16
        in_maps.append({"xT": shard_T, "A": A, "d": d})

    if _PROGRAM is None:
        _PROGRAM = _build_program()

    kres = bass_utils.run_bass_kernel_spmd(
        _PROGRAM, in_maps, core_ids=list(range(N_CORES)), trace=trace
    )

    parts = []
    for i in range(N_CORES):
        o = np.asarray(kres.results[i]["out"])  # [NQ, 128, NB, T]
        # s = q*QW + j*GW + 32*blk + r with p = 32*j + r
        o = o.reshape(NQ, 4, 32, NB, T)  # q, j, r, blk, t
        parts.append(o.transpose(4, 0, 1, 3, 2).reshape(T, BS))
    full = np.concatenate(parts, axis=1)[:, :, None].astype(np.float32)
    return full, kres


def kernel(**inputs):
    out, _ = _run(inputs, trace=bool(int(os.environ.get("KERNEL_TRACE", "0"))))
    return out


# revision 10
# speedup vs baseline: 1.0703x; 1.0443x over previous
"""Trainium2 Bass kernel for the CGC multi-task MoE routing module.

Math: the reference computes, per task t:
    expert outputs  E[t,e] = x @ W[t,e] + b[t,e]          (ES specific + EC common)
    gate logits     L[t]   = concat_e(E[t,e]) @ Wg[t] + bg[t]
    weights         p      = softmax(L[t])
    feature         F[t]   = sum_e p_e * E[t,e]
    out[t]          = F[t] @ Wt[t] + bt[t]                # scalar per sample

Both L[t] and the per-expert scalars s[t,e] = E[t,e] @ Wt[t] are linear in x,
so everything folds into one skinny matmul z = x @ A + d with
A: [I, 24] (per task: 6 logit cols + 6 scalar cols), followed by a per-sample
6-way softmax-weighted average:
    out[t,b] = sum_e exp(L_e) * s_e / sum_e exp(L_e)      (bt folded into s).

v4: x and A are bf16 (tolerance 2e-2; bf16 dot error ~2e-3), halving HBM
traffic.  x loads are all issued up-front, alternating between the two
HWDGE rings (sync + scalar); A/d ride gpsimd SWDGE.  The final band is
split into 128-sample pieces so the post-stream serial chain
(matmul->bias->transpose->exp->reduce->divide->store) runs on tiny
free dims.  The softmax-average uses a fused elementwise divide.
"""

import os

import numpy as np

B, I, H = 65536, 512, 128
T, ES, EC = 2, 2, 4
ETOT = ES + EC

N_CORES = 8
BS = B // N_CORES  # samples per core
M = 32  # folded output channels, padded 24 -> 32 for the 32x32 transpose
GW = 512  # samples per band (one PSUM bank per band)
QW = 4 * GW  # samples per quad (4 bands stacked on the 128 partitions)
NQ = BS // QW
NCHUNK = I // 128
NB = GW // 32  # 32-sample blocks per band

# x loads (samples): 1MB loads for bands 0..13, then a taper so the last
# pieces complete (and can be post-processed) with minimal serial latency
LOADS = [
    (0, 1024),
    (1024, 1024),
    (2048, 1024),
    (3072, 1024),
    (4096, 1024),
    (5120, 1024),
    (6144, 1024),
    (7168, 512),
    (7680, 256),
    (7936, 128),
    (8064, 128),
]
assert sum(n for _, n in LOADS) == BS


def _fold(inputs):
    """Fold all weights into A [128, NCHUNK, M] (bf16) and bias d [M,1] (f32).

    Channel layout per task t (base 12*t): 0:6 gate logits, 6:12 per-expert
    scalars (bt folded in, valid since softmax weights sum to 1).
    A is packed so that partition p, chunk c holds row c*128+p of the
    [I, M] matrix (matching the xT chunk view).
    """
    import ml_dtypes

    w64 = lambda k: np.asarray(inputs[k], np.float64)
    Wc, bc, Ws, bs = w64("Wc"), w64("bc"), w64("Ws"), w64("bs")
    Wg, bg, Wt, bt = w64("Wg"), w64("bg"), w64("Wt"), w64("bt")

    A = np.zeros((I, M))
    d = np.zeros(M)
    for t in range(T):
        W_all = np.concatenate(
            [Ws[t, e] for e in range(ES)] + [Wc[e] for e in range(EC)], axis=1
        )  # [I, ETOT*H]
        b_all = np.concatenate(
            [bs[t, e] for e in range(ES)] + [bc[e] for e in range(EC)]
        )  # [ETOT*H]
        A[:, 12 * t : 12 * t + 6] = W_all @ Wg[t]
        d[12 * t : 12 * t + 6] = b_all @ Wg[t] + bg[t]
        A[:, 12 * t + 6 : 12 * t + 12] = (
            W_all.reshape(I, ETOT, H) * Wt[t, :, 0][None, None, :]
        ).sum(-1)
        d[12 * t + 6 : 12 * t + 12] = (
            b_all.reshape(ETOT, H) * Wt[t, :, 0][None, :]
        ).sum(-1) + bt[t, 0]
    Apack = (
        A.reshape(NCHUNK, 128, M).transpose(1, 0, 2).astype(ml_dtypes.bfloat16)
    )  # [128, NCHUNK, M]
    return np.ascontiguousarray(Apack), d.reshape(M, 1).astype(np.float32)


def _build_program():
    import concourse.bacc as bacc
    import concourse.mybir as mybir
    from concourse.tile import TileContext

    f32 = mybir.dt.float32
    bf16 = mybir.dt.bfloat16

    nc = bacc.Bacc("TRN2", target_bir_lowering=False, debug=False, num_devices=N_CORES)
    xT_ext = nc.declare_dram_parameter("xT", [I, BS], bf16, isOutput=False)
    A_ext = nc.declare_dram_parameter("A", [128, NCHUNK, M], bf16, isOutput=False)
    d_ext = nc.declare_dram_parameter("d", [M, 1], f32, isOutput=False)
    # out[q, p, blk, t]: sample s = q*QW + (p//32)*GW + 32*blk + p%32, task t
    out_ext = nc.declare_dram_parameter("out", [NQ, 128, NB, T], f32, isOutput=True)

    xT_view = xT_ext[:, :].rearrange("(c p) b -> p c b", p=128)  # [128, NCHUNK, BS]

    with TileContext(nc) as tc:
        with (
            tc.tile_pool(name="consts", bufs=1) as cpool,
            tc.tile_pool(name="xin", bufs=1) as xpool,
            tc.tile_pool(name="zt", bufs=3) as ztpool,
            tc.tile_pool(name="zq", bufs=3) as zqpool,
            tc.tile_pool(name="epi", bufs=4) as epool,
            tc.tile_pool(name="psum", bufs=2, space="PSUM") as ppool,
        ):
            # A/d ride the gpsimd SWDGE; scalar stays compute-only (a DMA
            # issue queued before the bias-adds would head-of-line-block them)
            A_sb = cpool.tile([128, NCHUNK, M], bf16)
            nc.gpsimd.dma_start(out=A_sb[:], in_=A_ext[:, :, :])
            d_sb = cpool.tile([M, 1], f32)
            nc.gpsimd.dma_start(out=d_sb[:], in_=d_ext[:, :])

            # prefetch the whole shard on the sync HWDGE ring; L1/L3 ride the
            # scalar ring (issued well before any bias-add queues behind them)
            # so two descriptor streams feed the SDMA engines during the ramp
            xs = []
            for k, (s0, n) in enumerate(LOADS):
                xk = xpool.tile([128, NCHUNK, n], bf16, name=f"x_{k}", tag=f"x{k}")
                eng = nc.scalar if k in (1, 3) else nc.sync
                eng.dma_start(out=xk[:], in_=xT_view[:, :, s0 : s0 + n])
                xs.append(xk)

            # band b (0..14) -> (load tile, sample offset within tile)
            def band_src(b):
                if b < 14:
                    return xs[b // 2], (b % 2) * GW
                return xs[7], 0  # band 14

            # piece p (0..3) of band 15 -> (load tile, offset)
            def piece_src(p):
                return [(xs[8], 0), (xs[8], 128), (xs[9], 0), (xs[10], 0)][p]

            from collections import deque

            pending = deque()

            def emit(fn):
                pending.append(fn)
                if len(pending) > 1:
                    pending.popleft()()

            ActT = mybir.ActivationFunctionType
            AxX = mybir.AxisListType.X
            AluAdd = mybir.AluOpType.add
            AluDiv = mybir.AluOpType.divide

            def mm_band(ps_ap, src, c_outer_items=None, *, cols):
                pass  # unused helper placeholder

            def epilogue(idx, Z, npart, nblk, res_t, store_fn):
                """softmax-weighted average on Z [npart, nblk*32]; writes
                res_t [npart, nblk, T]; then store_fn() if given."""
                Zb = Z.rearrange("p (blk c) -> p blk c", c=32)
                zt4 = Zb[:, :, 0:24].rearrange("p blk (t c) -> p blk t c", c=12)
                lg = zt4[:, :, :, 0:6]
                sc = zt4[:, :, :, 6:12]
                ssum = epool.tile(
                    [npart, nblk, T], f32, name=f"ssum_{idx}", tag="ssum"
                )
                num = epool.tile([npart, nblk, T], f32, name=f"num_{idx}", tag="num")
                rinv = epool.tile(
                    [npart, nblk, T], f32, name=f"rinv_{idx}", tag="rinv"
                )

                nc.scalar.activation(lg, lg, ActT.Exp)
                nc.vector.tensor_reduce(ssum[:], lg, axis=AxX, op=AluAdd)
                nc.vector.tensor_mul(lg, lg, sc)  # exp * s, clobbers exp
                nc.vector.tensor_reduce(num[:], lg, axis=AxX, op=AluAdd)
                nc.vector.reciprocal(rinv[:], ssum[:])
                nc.vector.tensor_mul(res_t, num[:], rinv[:])
                if store_fn is not None:
                    store_fn()

            # ---- quads 0..2: full 128-partition pipeline ----
            for q in range(3):
                zT_sb = ztpool.tile([128, GW], f32, name=f"zTsb_{q}", tag="zTsb")
                pss = [
                    ppool.tile([M, GW], f32, name=f"ps_{q}_{j}", tag=f"ps{j}")
                    for j in range(4)
                ]
                # chunk-outer: consecutive matmuls share stationary weights
                for c in range(NCHUNK):
                    for j in range(4):
                        xk, off = band_src(4 * q + j)
                        nc.tensor.matmul(
                            pss[j][:, :],
                            A_sb[:, c, :],
                            xk[:, c, off : off + GW],
                            start=(c == 0),
                            stop=(c == NCHUNK - 1),
                        )
                # PSUM -> SBUF band writes with per-partition bias add,
                # split across scalar and vector (gpsimd can't read PSUM)
                for j in range(4):
                    dst = zT_sb[32 * j : 32 * j + 32, :]
                    if j < 3:
                        nc.scalar.add(dst, pss[j][:, :], d_sb[:])
                    else:
                        nc.vector.tensor_scalar_add(dst, pss[j][:, :], d_sb[:])

                Zq = zqpool.tile([128, GW], f32, name=f"Z_{q}", tag="Z")
                nc.vector.transpose(Zq[:], zT_sb[:])

                def epi_quad(q=q, Zq=Zq):
                    res = epool.tile([128, NB, T], f32, name=f"res_{q}", tag="res")
                    epilogue(
                        f"q{q}",
                        Zq[:],
                        128,
                        NB,
                        res[:],
                        lambda: nc.sync.dma_start(
                            out=out_ext[q, :, :, :], in_=res[:]
                        ),
                    )

                emit(epi_quad)

            # ---- bands 12,13: 64-partition group ----
            zT34 = ztpool.tile([64, GW], f32, name="zT34", tag="zt34", bufs=1)
            ps34 = [
                ppool.tile([M, GW], f32, name=f"ps34_{j}", tag=f"ps{j}")
                for j in range(2)
            ]
            for j in range(2):
                xk, off = band_src(12 + j)
                for c in range(NCHUNK):
                    nc.tensor.matmul(
                        ps34[j][:, :],
                        A_sb[:, c, :],
                        xk[:, c, off : off + GW],
                        start=(c == 0),
                        stop=(c == NCHUNK - 1),
                    )
                dst = zT34[32 * j : 32 * j + 32, :]
                if j == 0:
                    nc.scalar.add(dst, ps34[j][:, :], d_sb[:])
                else:
                    nc.vector.tensor_scalar_add(dst, ps34[j][:, :], d_sb[:])
            Zq34 = zqpool.tile([64, GW], f32, name="Zq34", tag="z34", bufs=1)
            nc.vector.transpose(Zq34[:], zT34[:])

            def epi_34(Zq34=Zq34):
                res = epool.tile([64, NB, T], f32, name="res_34", tag="res")
                epilogue(
                    "g34",
                    Zq34[:],
                    64,
                    NB,
                    res[:],
                    lambda: nc.sync.dma_start(
                        out=out_ext[3, 0:64, :, :], in_=res[:]
                    ),
                )

            emit(epi_34)

            # ---- band 14: 32-partition group ----
            zT14 = ztpool.tile([M, GW], f32, name="zT14", tag="zt14", bufs=1)
            ps14 = ppool.tile([M, GW], f32, name="ps14", tag="ps2")
            xk, off = band_src(14)
            for c in range(NCHUNK):
                nc.tensor.matmul(
                    ps14[:, :],
                    A_sb[:, c, :],
                    xk[:, c, off : off + GW],
                    start=(c == 0),
                    stop=(c == NCHUNK - 1),
                )
            nc.scalar.add(zT14[:, :], ps14[:, :], d_sb[:])
            Zq14 = zqpool.tile([M, GW], f32, name="Zq14", tag="z14", bufs=1)
            nc.vector.transpose(Zq14[:], zT14[:])

            def epi_14(Zq14=Zq14):
                res = epool.tile([M, NB, T], f32, name="res_14", tag="res")
                epilogue(
                    "g14",
                    Zq14[:],
                    32,
                    NB,
                    res[:],
                    lambda: nc.sync.dma_start(
                        out=out_ext[3, 64:96, :, :], in_=res[:]
                    ),
                )

            emit(epi_14)

            # ---- band 15: four 128-sample pieces (short tail chain) ----
            PW = 128  # piece width
            zT15 = ztpool.tile([M, GW], f32, name="zT15", tag="zt15", bufs=1)
            ps15 = ppool.tile([M, GW], f32, name="ps15", tag="ps3")
            Zq15 = zqpool.tile([M, GW], f32, name="Zq15", tag="z15", bufs=1)
            res15 = epool.tile([M, NB, T], f32, name="res_15", tag="res15", bufs=1)
            for p in range(4):
                xk, off = piece_src(p)
                pslice = ps15[:, PW * p : PW * (p + 1)]
                for c in range(NCHUNK):
                    nc.tensor.matmul(
                        pslice,
                        A_sb[:, c, :],
                        xk[:, c, off : off + PW],
                        start=(c == 0),
                        stop=(c == NCHUNK - 1),
                    )
                dst = zT15[:, PW * p : PW * (p + 1)]
                if p % 2 == 0:
                    nc.scalar.add(dst, pslice, d_sb[:])
                else:
                    nc.vector.tensor_scalar_add(dst, pslice, d_sb[:])
                nc.vector.transpose(
                    Zq15[:, PW * p : PW * (p + 1)], zT15[:, PW * p : PW * (p + 1)]
                )

                def epi_piece(p=p):
                    store = None
                    if p == 3:
                        store = lambda: nc.sync.dma_start(
                            out=out_ext[3, 96:128, :, :], in_=res15[:]
                        )
                    epilogue(
                        f"p{p}",
                        Zq15[:, PW * p : PW * (p + 1)],
                        32,
                        PW // 32,
                        res15[:, 4 * p : 4 * (p + 1), :],
                        store,
                    )

                emit(epi_piece)

            while pending:
                pending.popleft()()

    nc.compile()
    return nc


_PROGRAM = None


def _ensure_ntff_hook():
    """Provide antenv.axon_hooks if the image lacks it (NTFF profiling)."""
    try:
        import antenv.axon_hooks  # noqa: F401

        return
    except ImportError:
        pass
    import contextlib
    import ctypes
    import sys
    import types

    import antenv

    mod = types.ModuleType("antenv.axon_hooks")
    holder = {"hook": None}
    mod.set_axon_ntff_profile_hook = lambda h: holder.__setitem__("hook", h)
    mod.get_axon_ntff_profile_hook = lambda: holder["hook"]
    sys.modules["antenv.axon_hooks"] = mod
    antenv.axon_hooks = mod

    so_path = "/opt/axon/libaxon_pjrt.so"
    try:
        lib = ctypes.CDLL(so_path)
    except OSError:
        return
    if not hasattr(lib, "axon_start_nrt_profile"):
        return
    lib.axon_start_nrt_profile.argtypes = [
        ctypes.POINTER(ctypes.c_int64),
        ctypes.c_size_t,
    ]
    lib.axon_start_nrt_profile.restype = ctypes.c_int64
    lib.axon_stop_nrt_profile.argtypes = [ctypes.c_char_p]
    lib.axon_stop_nrt_profile.restype = ctypes.c_int64

    @contextlib.contextmanager
    def _hook(output_dir, device_ids):
        import jax

        jax.devices()
        if device_ids:
            ids = (ctypes.c_int64 * len(device_ids))(*device_ids)
            rc = lib.axon_start_nrt_profile(ids, len(device_ids))
        else:
            rc = lib.axon_start_nrt_profile(None, 0)
        if rc != 0:
            raise RuntimeError(f"axon_start_nrt_profile rc={rc}")
        try:
            yield
        finally:
            n = lib.axon_stop_nrt_profile(str(output_dir).encode())
            print(f"ntff profile: {n} file(s) written to {output_dir}")

    mod.set_axon_ntff_profile_hook(_hook)


def _run(inputs, trace=False):
    global _PROGRAM
    import ml_dtypes

    import concourse.bass_utils as bass_utils

    if trace:
        _ensure_ntff_hook()
        # keep trace artifacts local; no bucket in this sandbox
        bass_utils.upload_artifacts = lambda tmpdir: "local://" + tmpdir

    A, d = _fold(inputs)
    x = np.asarray(inputs["x"], np.float32)
    in_maps = []
    for i in range(N_CORES):
        shard_T = np.ascontiguousarray(
            x[i * BS : (i + 1) * BS].T.astype(ml_dtypes.bfloat16)
        )  # [I, BS] bf16
        in_maps.append({"xT": shard_T, "A": A, "d": d})

    if _PROGRAM is None:
        _PROGRAM = _build_program()

    kres = bass_utils.run_bass_kernel_spmd(
        _PROGRAM, in_maps, core_ids=list(range(N_CORES)), trace=trace
    )

    parts = []
    for i in range(N_CORES):
        o = np.asarray(kres.results[i]["out"])  # [NQ, 128, NB, T]
        # s = q*QW + j*GW + 32*blk + r with p = 32*j + r
        o = o.reshape(NQ, 4, 32, NB, T)  # q, j, r, blk, t
        parts.append(o.transpose(4, 0, 1, 3, 2).reshape(T, BS))
    full = np.concatenate(parts, axis=1)[:, :, None].astype(np.float32)
    return full, kres


def kernel(**inputs):
    out, _ = _run(inputs, trace=bool(int(os.environ.get("KERNEL_TRACE", "0"))))
    return out
